# revision 1
# baseline (speedup 1.0000x reference)
"""LocallyConnected1D (B=8, L=4096, C=64, K=3, F=64) on 8 TRN2 NeuronCores.

out[b, l, f] = sum_{k,c} x[b, l+k, c] * kernel[l, k, c, f] + bias[l, f]

Strategy (spatial sharding, 512 output positions per core):
  - For each pair of adjacent output positions (l0+2i, l0+2i+1) build a
    block-diagonal stationary tile lhsT (128 x 16): partitions = 2 phases x 64
    channels, columns = 2 phases x 8 batch.  Streaming operand = the pair's
    per-position weights (128 x 64).  Three PSUM-accumulated matmuls per pair
    (one per tap k, using x-pair tiles shifted by k) produce out (16, 64).
  - Groups of 8 pairs are dispatched to 4 independent 32-column strips of the
    PE array (tile_position), each strip accumulating into its own PSUM bank,
    so up to 4 matmuls run concurrently in the array.
  - Weights AND x-pair tiles are packed into one contiguous DRAM blob per
    block -> dense DMAs at full HBM bandwidth.  First blocks are small so the
    PE starts early; per-block outputs go out in a single DMA.
  - Compute in bf16 (PSUM accumulation in f32); bias added on host.
"""

import numpy as np
import ml_dtypes

import concourse.bass as bass
import concourse.mybir as mybir
import concourse.tile as tile
from concourse import bacc
from concourse.bass import ds, ts
from concourse.bass_utils import run_bass_kernel_spmd

B, L, C, K, F = 8, 4096, 64, 3, 64
L_OUT = (L - K) + 1  # 4094
N_CORES = 8
P_CORE = 512          # output positions per core (last core: 510 real + 2 pad)
PAIRS = P_CORE // 2   # 256

# pairs per DMA block; small first blocks let the PE start early
BLOCKS = [8, 8, 16] + [32] * 6 + [16, 8, 8]
assert sum(BLOCKS) == PAIRS and all(b % 8 == 0 for b in BLOCKS)

USE_BF16 = True
DT = mybir.dt.bfloat16 if USE_BF16 else mybir.dt.float32
NPDT = ml_dtypes.bfloat16 if USE_BF16 else np.float32
DT_OUT = mybir.dt.float32

# per-block columns (per partition): weights | te tiles | to tiles
def _blk_cols(n):
    return n * K * F + (n + 1) * 16 + n * 16

BLK_OFF = np.cumsum([0] + [_blk_cols(n) for n in BLOCKS]).tolist()
TOT_COLS = BLK_OFF[-1]

_CACHE = {}


def _build_body(nc, wpool, opool, pspool, blk_d, out_d):
    s = 0  # first pair of current block
    for h, n in enumerate(BLOCKS):
        cols = _blk_cols(n)
        blk = wpool.tile([128, cols], DT, name="blk", tag="blk",
                         padded_shape=[128, _blk_cols(max(BLOCKS))])
        nc.sync.dma_start(blk[:], blk_d[:, ds(BLK_OFF[h], cols)])
        w_cols = n * K * F
        te_cols = (n + 1) * 16
        ngroups = n // 8
        accs = [pspool.tile([128, 512], DT_OUT, name=f"acc{q}", tag=f"acc{q}")
                for q in range(ngroups)]

        def te_ap(i):   # block-diag tile for even-start pair i
            return blk[:, ds(w_cols + (i - s) * 16, 16)]

        def to_ap(i):   # odd-start pair i
            return blk[:, ds(w_cols + te_cols + (i - s) * 16, 16)]

        def w_ap(jj, k):
            return blk[:, ds((jj * K + k) * F, F)]

        for j in range(8):
            for q in range(ngroups):
                i = s + q * 8 + j   # global pair
                jj = q * 8 + j      # pair in block
                o_ap = accs[q][ds(32 * q, 16), ts(j, 64)]
                tp = (0, 32 * q)
                nc.tensor.matmul(o_ap, te_ap(i), w_ap(jj, 0),
                                 start=True, stop=False, tile_position=tp)
                nc.tensor.matmul(o_ap, to_ap(i), w_ap(jj, 1),
                                 start=False, stop=False, tile_position=tp)
                nc.tensor.matmul(o_ap, te_ap(i + 1), w_ap(jj, 2),
                                 start=False, stop=True, tile_position=tp)
        ob = opool.tile([16, ngroups * 512], DT_OUT, name="ob", tag="ob",
                        padded_shape=[16, 4 * 512])
        for q in range(ngroups):
            nc.vector.tensor_copy(ob[:, ds(q * 512, 512)],
                                  accs[q][ds(32 * q, 16), :])
        g0 = s // 8  # first global group of this block
        nc.scalar.dma_start(out_d[:, ds(g0 * 512, ngroups * 512)], ob[:])
        s += n


def _build_nc(n_iters=None):
    """n_iters=None: straight-line kernel (graded path).
    n_iters=N: body wrapped in a HW For_i loop, for timing-slope runs."""
    nc = bacc.Bacc("TRN2", target_bir_lowering=False, debug=False)

    blk_d = nc.declare_dram_parameter("blk", [128, TOT_COLS], DT, isOutput=False)
    # out[m, g*512 + j*64 + f]: g = group of 8 pairs, m = phase*8 + b.
    out_d = nc.declare_dram_parameter("out", [16, (PAIRS // 8) * 512], DT_OUT,
                                      isOutput=True)

    with tile.TileContext(nc) as tc:
        with (
            tc.tile_pool(name="wpool", bufs=8) as wpool,
            tc.tile_pool(name="opool", bufs=8) as opool,
            # 4 acc tags (one per PE strip) x 2 bufs = all 8 PSUM banks
            tc.tile_pool(name="pspool", bufs=2, space=bass.MemorySpace.PSUM) as pspool,
        ):
            if n_iters is None:
                _build_body(nc, wpool, opool, pspool, blk_d, out_d)
            else:
                with tc.For_i(0, n_iters, 1):
                    _build_body(nc, wpool, opool, pspool, blk_d, out_d)

    nc.compile()
    return nc


def _prep_inputs(x, kernel):
    """Host-side rearrangement into per-core fused block layouts."""
    xp = np.zeros((B, L + 4, C), np.float32)
    xp[:, :L] = x
    kp = np.zeros((N_CORES * P_CORE, K, C, F), np.float32)
    kp[:L_OUT] = kernel
    in_maps = []
    for m in range(N_CORES):
        l0 = P_CORE * m
        xs = xp[:, l0:l0 + 2 * PAIRS + 2, :]
        ev = xs[:, 0::2].transpose(2, 1, 0)  # (64, 257, 8)  j = 2i
        od = xs[:, 1::2].transpose(2, 1, 0)  # (64, 257, 8)  j = 2i+1
        # TE[i]: pair (2i, 2i+1); TO[i]: pair (2i+1, 2i+2); block-diag (128,16)
        TE = np.zeros((128, PAIRS + 1, 16), np.float32)
        TE[:64, :, 0:8] = ev
        TE[64:, :, 8:16] = od
        TO = np.zeros((128, PAIRS, 16), np.float32)
        TO[:64, :, 0:8] = od[:, :PAIRS]
        TO[64:, :, 8:16] = ev[:, 1:PAIRS + 1]
        W = (kp[l0:l0 + P_CORE]
             .reshape(PAIRS, 2, K, C, F)
             .transpose(1, 3, 0, 2, 4)
             .reshape(128, PAIRS, K, F))  # [pc, pair, k, f]
        blk = np.empty((128, TOT_COLS), np.float32)
        s = 0
        for h, n in enumerate(BLOCKS):
            o = BLK_OFF[h]
            w_cols = n * K * F
            blk[:, o:o + w_cols] = W[:, s:s + n].reshape(128, w_cols)
            blk[:, o + w_cols:o + w_cols + (n + 1) * 16] = (
                TE[:, s:s + n + 1].reshape(128, (n + 1) * 16))
            blk[:, o + w_cols + (n + 1) * 16:o + _blk_cols(n)] = (
                TO[:, s:s + n].reshape(128, n * 16))
            s += n
        in_maps.append({"blk": blk.astype(NPDT)})
    return in_maps


def _unpack_out(res):
    """(16, 32*512) per core -> (B, P_CORE, F).  l_local = 16g + 2j + phase."""
    return (res.reshape(2, 8, 32, 8, 64)          # [phase, b, g, j, f]
            .transpose(1, 2, 3, 0, 4)              # [b, g, j, phase, f]
            .reshape(B, P_CORE, F))


def kernel(x, kernel, bias):
    x = np.asarray(x, dtype=np.float32)
    kern = np.asarray(kernel, dtype=np.float32)
    bias = np.asarray(bias, dtype=np.float32)

    if "nc" not in _CACHE:
        _CACHE["nc"] = _build_nc()
    nc = _CACHE["nc"]

    in_maps = _prep_inputs(x, kern)
    results = run_bass_kernel_spmd(nc, in_maps, list(range(N_CORES))).results

    parts = [_unpack_out(results[m]["out"]) for m in range(N_CORES)]
    out = np.concatenate(parts, axis=1)[:, :L_OUT]
    return (out + bias[None]).astype(np.float32)



# revision 2
# speedup vs baseline: 1.3289x; 1.3289x over previous
"""LocallyConnected1D (B=8, L=4096, C=64, K=3, F=64) on 8 TRN2 NeuronCores.

out[b, l, f] = sum_{k,c} x[b, l+k, c] * kernel[l, k, c, f] + bias[l, f]

Strategy (spatial sharding, 512 output positions per core):
  - For each pair of adjacent output positions (l0+2i, l0+2i+1) build a
    block-diagonal stationary tile lhsT (128 x 16): partitions = 2 phases x 64
    channels, columns = 2 phases x 8 batch.  Streaming operand = the pair's
    per-position weights (128 x 64).  Three PSUM-accumulated matmuls per pair
    (one per tap k, using x-pair tiles shifted by k) produce out (16, 64).
  - Groups of 8 pairs are dispatched to 4 independent 32-column strips of the
    PE array (tile_position), each strip accumulating into its own PSUM bank,
    so up to 4 matmuls run concurrently in the array.
  - The kernel is HBM-bound: the per-position weights (~100 MB total) are
    each used exactly once.  Weights ship as fp8 e3m4 (scaled x16 on host;
    4 mantissa bits keep max-rel error ~1.4e-2 < 2e-2), halving the dominant
    DMA stream vs bf16.  x pair-tiles stay bf16; outputs return as bf16 and
    are upscaled (exact /16) on host.  PSUM accumulates in f32.
  - Weights and x tiles are packed into contiguous per-block DRAM blobs ->
    dense large DMAs.  First blocks are small so the PE starts early.
"""

import numpy as np
import ml_dtypes

import concourse.bass as bass
import concourse.mybir as mybir
import concourse.tile as tile
from concourse import bacc
from concourse.bass import ds, ts
from concourse.bass_utils import run_bass_kernel_spmd

B, L, C, K, F = 8, 4096, 64, 3, 64
L_OUT = (L - K) + 1  # 4094
N_CORES = 8
P_CORE = 512          # output positions per core (last core: 510 real + 2 pad)
PAIRS = P_CORE // 2   # 256

# pairs per DMA block; small first blocks let the PE start early
BLOCKS = [8, 8, 16] + [32] * 6 + [16, 8, 8]
assert sum(BLOCKS) == PAIRS and all(b % 8 == 0 for b in BLOCKS)

WSCALE = 16.0  # host-side weight prescale before fp8 e3m4 cast
DT_W = mybir.dt.float8e3
NP_W = ml_dtypes.float8_e3m4
DT_X = mybir.dt.bfloat16
NP_X = ml_dtypes.bfloat16
DT_OUT = mybir.dt.bfloat16


def _w_cols(n):
    return n * K * F


def _x_cols(n):
    return (2 * n + 1) * 16


W_OFF = np.cumsum([0] + [_w_cols(n) for n in BLOCKS]).tolist()
X_OFF = np.cumsum([0] + [_x_cols(n) for n in BLOCKS]).tolist()
W_TOT = W_OFF[-1]
X_TOT = X_OFF[-1]

_CACHE = {}


def _build_body(nc, wpool, xpool, opool, pspool, w_d, x_d, out_d):
    s = 0  # first pair of current block
    for h, n in enumerate(BLOCKS):
        wblk = wpool.tile([128, _w_cols(n)], DT_W, name="wblk", tag="wblk",
                          padded_shape=[128, _w_cols(max(BLOCKS))])
        xblk = xpool.tile([128, _x_cols(n)], DT_X, name="xblk", tag="xblk",
                          padded_shape=[128, _x_cols(max(BLOCKS))])
        nc.sync.dma_start(wblk[:], w_d[:, ds(W_OFF[h], _w_cols(n))])
        nc.sync.dma_start(xblk[:], x_d[:, ds(X_OFF[h], _x_cols(n))])
        te_cols = (n + 1) * 16
        ngroups = n // 8
        accs = [pspool.tile([128, 512], mybir.dt.float32, name=f"acc{q}",
                            tag=f"acc{q}") for q in range(ngroups)]

        def te_ap(i):   # block-diag tile for even-start pair i
            return xblk[:, ds((i - s) * 16, 16)]

        def to_ap(i):   # odd-start pair i
            return xblk[:, ds(te_cols + (i - s) * 16, 16)]

        def w_ap(jj, k):
            return wblk[:, ds((jj * K + k) * F, F)]

        for j in range(8):
            for q in range(ngroups):
                i = s + q * 8 + j   # global pair
                jj = q * 8 + j      # pair in block
                o_ap = accs[q][ds(32 * q, 16), ts(j, 64)]
                tp = (0, 32 * q)
                nc.tensor.matmul(o_ap, te_ap(i), w_ap(jj, 0),
                                 start=True, stop=False, tile_position=tp)
                nc.tensor.matmul(o_ap, to_ap(i), w_ap(jj, 1),
                                 start=False, stop=False, tile_position=tp)
                nc.tensor.matmul(o_ap, te_ap(i + 1), w_ap(jj, 2),
                                 start=False, stop=True, tile_position=tp)
        ob = opool.tile([16, ngroups * 512], DT_OUT, name="ob", tag="ob",
                        padded_shape=[16, 4 * 512])
        for q in range(ngroups):
            nc.vector.tensor_copy(ob[:, ds(q * 512, 512)],
                                  accs[q][ds(32 * q, 16), :])
        g0 = s // 8  # first global group of this block
        nc.scalar.dma_start(out_d[:, ds(g0 * 512, ngroups * 512)], ob[:])
        s += n


def _build_nc(n_iters=None):
    """n_iters=None: straight-line kernel (graded path).
    n_iters=N: body wrapped in a HW For_i loop, for timing-slope runs."""
    nc = bacc.Bacc("TRN2", target_bir_lowering=False, debug=False)

    w_d = nc.declare_dram_parameter("wd", [128, W_TOT], DT_W, isOutput=False)
    x_d = nc.declare_dram_parameter("xd", [128, X_TOT], DT_X, isOutput=False)
    # out[m, g*512 + j*64 + f]: g = group of 8 pairs, m = phase*8 + b.
    out_d = nc.declare_dram_parameter("out", [16, (PAIRS // 8) * 512], DT_OUT,
                                      isOutput=True)

    with tile.TileContext(nc) as tc:
        with (
            tc.tile_pool(name="wpool", bufs=8) as wpool,
            tc.tile_pool(name="xpool", bufs=8) as xpool,
            tc.tile_pool(name="opool", bufs=8) as opool,
            # 4 acc tags (one per PE strip) x 2 bufs = all 8 PSUM banks
            tc.tile_pool(name="pspool", bufs=2, space=bass.MemorySpace.PSUM) as pspool,
        ):
            if n_iters is None:
                _build_body(nc, wpool, xpool, opool, pspool, w_d, x_d, out_d)
            else:
                with tc.For_i(0, n_iters, 1):
                    _build_body(nc, wpool, xpool, opool, pspool, w_d, x_d, out_d)

    nc.compile()
    return nc


def _prep_inputs(x, kernel):
    """Host-side rearrangement into per-core fused block layouts."""
    xp = np.zeros((B, L + 4, C), np.float32)
    xp[:, :L] = x
    kp = np.zeros((N_CORES * P_CORE, K, C, F), np.float32)
    kp[:L_OUT] = kernel
    in_maps = []
    for m in range(N_CORES):
        l0 = P_CORE * m
        xs = xp[:, l0:l0 + 2 * PAIRS + 2, :]
        ev = xs[:, 0::2].transpose(2, 1, 0)  # (64, 257, 8)  j = 2i
        od = xs[:, 1::2].transpose(2, 1, 0)  # (64, 257, 8)  j = 2i+1
        # TE[i]: pair (2i, 2i+1); TO[i]: pair (2i+1, 2i+2); block-diag (128,16)
        TE = np.zeros((128, PAIRS + 1, 16), np.float32)
        TE[:64, :, 0:8] = ev
        TE[64:, :, 8:16] = od
        TO = np.zeros((128, PAIRS, 16), np.float32)
        TO[:64, :, 0:8] = od[:, :PAIRS]
        TO[64:, :, 8:16] = ev[:, 1:PAIRS + 1]
        W = (kp[l0:l0 + P_CORE]
             .reshape(PAIRS, 2, K, C, F)
             .transpose(1, 3, 0, 2, 4)
             .reshape(128, PAIRS, K, F))  # [pc, pair, k, f]
        wb = np.empty((128, W_TOT), np.float32)
        xb = np.empty((128, X_TOT), np.float32)
        s = 0
        for h, n in enumerate(BLOCKS):
            wb[:, W_OFF[h]:W_OFF[h] + _w_cols(n)] = (
                W[:, s:s + n].reshape(128, _w_cols(n)))
            xo = X_OFF[h]
            xb[:, xo:xo + (n + 1) * 16] = (
                TE[:, s:s + n + 1].reshape(128, (n + 1) * 16))
            xb[:, xo + (n + 1) * 16:xo + _x_cols(n)] = (
                TO[:, s:s + n].reshape(128, n * 16))
            s += n
        in_maps.append({
            "wd": (wb * WSCALE).astype(NP_W),
            "xd": xb.astype(NP_X),
        })
    return in_maps


def _unpack_out(res):
    """(16, 32*512) per core -> (B, P_CORE, F).  l_local = 16g + 2j + phase."""
    return (res.astype(np.float32)
            .reshape(2, 8, 32, 8, 64)              # [phase, b, g, j, f]
            .transpose(1, 2, 3, 0, 4)              # [b, g, j, phase, f]
            .reshape(B, P_CORE, F))


def kernel(x, kernel, bias):
    x = np.asarray(x, dtype=np.float32)
    kern = np.asarray(kernel, dtype=np.float32)
    bias = np.asarray(bias, dtype=np.float32)

    if "nc" not in _CACHE:
        _CACHE["nc"] = _build_nc()
    nc = _CACHE["nc"]

    in_maps = _prep_inputs(x, kern)
    results = run_bass_kernel_spmd(nc, in_maps, list(range(N_CORES))).results

    parts = [_unpack_out(results[m]["out"]) for m in range(N_CORES)]
    out = np.concatenate(parts, axis=1)[:, :L_OUT] * (1.0 / WSCALE)
    return (out + bias[None]).astype(np.float32)


# revision 14
# speedup vs baseline: 1.4215x; 1.0697x over previous
"""LocallyConnected1D (B=8, L=4096, C=64, K=3, F=64) on 8 TRN2 NeuronCores.

out[b, l, f] = sum_{k,c} x[b, l+k, c] * kernel[l, k, c, f] + bias[l, f]

Strategy (spatial sharding, 512 output positions per core):
  - Pairs of adjacent output positions (2i, 2i+1): stationary tile TE[i]
    (128 x 16) = block-diag(x[2i], x[2i+1]) over (2 phases x 64 channels)
    partitions; streaming operand = per-position fp8 weights.  TO[i] =
    block-diag(x[2i+1], x[2i+2]) serves tap 1.
  - Per pair: tap1 (TO, 64-col matmul) opens the pair, then TE matmuls
    accumulate.  TE[j] serves pair j-1 tap2 AND pair j tap0 whose weight and
    PSUM columns are adjacent -> fused 128-col matmuls (17 MM per group of 8
    pairs instead of 24).  PSUM has_written bits: only the block's first MM
    uses start=True (clears the whole bank); bit=0 -> overwrite handles each
    fresh region after that.
  - All 4 groups of a 32-pair block accumulate into ONE PSUM bank (strip q ->
    partitions 32q..32q+16, tile_position col 32q) so a single full-width
    [128, 512] DVE copy drains the block (vs 4 thin 16-row copies).
  - HBM traffic minimized: weights ship as fp8 e3m4 (x16 host prescale, ~1.4e-2
    max-rel error), x ships as bf16 TE tiles only; TO tiles are built on-chip
    by two partition-shifted DVE copies whose column shift (+8) picks up TE's
    own zero quadrants, so no memsets are needed.  Output returns as bf16.
  - Per block: 2 input DMAs (sync ring), 1 output DMA (scalar ring) -> the
    ~600ns-per-DMA sequencer issue cost stays off the critical path.
"""

import numpy as np
import ml_dtypes

import concourse.bass as bass
import concourse.mybir as mybir
import concourse.tile as tile
from concourse import bacc
from concourse.bass import ds, ts
from concourse.bass_utils import run_bass_kernel_spmd

B, L, C, K, F = 8, 4096, 64, 3, 64
L_OUT = (L - K) + 1  # 4094
N_CORES = 8
P_CORE = 512          # output positions per core (last core: 510 real + 2 pad)
PAIRS = P_CORE // 2   # 256

BLOCKS = [8, 8, 32, 32, 32, 32, 32, 32, 32, 16]   # pairs per block
assert sum(BLOCKS) == PAIRS and all(b % 8 == 0 for b in BLOCKS)
NB = len(BLOCKS)

WSCALE = 16.0  # host-side weight prescale before fp8 e3m4 cast
DT_W = mybir.dt.float8e3
NP_W = ml_dtypes.float8_e3m4
DT_X = mybir.dt.bfloat16
NP_X = ml_dtypes.bfloat16
DT_OUT = mybir.dt.bfloat16


def _w_cols(n):
    return n * K * F


def _te_cols(n):
    return (n + 1) * 16


W_OFF = np.cumsum([0] + [_w_cols(n) for n in BLOCKS]).tolist()
X_OFF = np.cumsum([0] + [_te_cols(n) for n in BLOCKS]).tolist()
W_TOT = W_OFF[-1]
X_TOT = X_OFF[-1]

_CACHE = {}


def _build_body(nc, wpool, xpool, opool, pspool, w_d, x_d, out_d):
    s = 0  # first pair of current block
    for h, n in enumerate(BLOCKS):
        te_c = _te_cols(n)
        to_c = n * 16
        wblk = wpool.tile([128, _w_cols(n)], DT_W, name="wblk", tag="wblk",
                          padded_shape=[128, _w_cols(max(BLOCKS))])
        xblk = xpool.tile([128, te_c + to_c], DT_X, name="xblk", tag="xblk",
                          padded_shape=[128, _te_cols(max(BLOCKS)) +
                                        max(BLOCKS) * 16])
        # Two HWDGE rings (SP=sync, ACT=scalar) each serialize their own
        # transfers; alternate w/x across them so each ring carries ~half the
        # input bytes.  Output DMAs ride the otherwise-idle SWDGE (gpsimd)
        # path so they never head-of-line-block an input ring.
        wq = nc.sync if h % 2 else nc.scalar
        xq = nc.scalar if h % 2 else nc.sync
        wq.dma_start(wblk[:], w_d[:, ds(W_OFF[h], _w_cols(n))])
        xq.dma_start(xblk[:, ds(0, te_c)], x_d[:, ds(X_OFF[h], te_c)])
        # Build TO tiles from TE tiles: TO[j] = blockdiag(od[j], ev[j+1]).
        # The +8 column shift reads TE's zero quadrants into TO's, so the
        # whole TO region (including zeros) is written.
        nc.vector.tensor_copy(xblk[ds(0, 64), ds(te_c, to_c)],
                              xblk[ds(64, 64), ds(8, to_c)])
        nc.vector.tensor_copy(xblk[ds(64, 64), ds(te_c, to_c)],
                              xblk[ds(0, 64), ds(8, to_c)])

        ngroups = n // 8
        acc = pspool.tile([128, 512], mybir.dt.float32, name="acc", tag="acc")

        def te_ap(i):   # block-diag tile for even-start pair i (global idx)
            return xblk[:, ds((i - s) * 16, 16)]

        def to_ap(i):   # odd-start pair i
            return xblk[:, ds(te_c + (i - s) * 16, 16)]

        def w_ap(jj, k, w=F):
            return wblk[:, ds((jj * K + k) * F, w)]

        # Per group (strip q): TO[0] TE[0] TO[1] TE[1] ... TO[7] TE[7] TE[8].
        # start=True only on the very first MM of the block (clears the whole
        # bank's has_written bits, stale from the pool's previous use).
        for step in range(17):
            for q in range(ngroups):
                g0 = s + q * 8      # first global pair of this group
                jj0 = q * 8         # first in-block pair of this group
                tp = (0, 32 * q)
                j, ph = divmod(step, 2)
                if ph == 0 and j < 8:     # TO[j]: pair j tap1, opens the pair
                    nc.tensor.matmul(acc[ds(32 * q, 16), ts(j, 64)],
                                     to_ap(g0 + j), w_ap(jj0 + j, 1),
                                     start=(step == 0), stop=False,
                                     tile_position=tp, skip_group_check=True)
                elif ph == 1 and j < 8:   # TE[j]
                    if j == 0:            # leading edge: pair 0 tap0 only
                        nc.tensor.matmul(acc[ds(32 * q, 16), ts(0, 64)],
                                         te_ap(g0), w_ap(jj0, 0),
                                         start=False, stop=False,
                                         tile_position=tp,
                                         skip_group_check=True)
                    else:                 # fused: pair j-1 tap2 | pair j tap0
                        nc.tensor.matmul(acc[ds(32 * q, 16),
                                             ds((j - 1) * 64, 128)],
                                         te_ap(g0 + j),
                                         w_ap(jj0 + j - 1, 2, 128),
                                         start=False, stop=False,
                                         tile_position=tp,
                                         skip_group_check=True)
                else:                     # step 16: trailing TE[8], pair7 tap2
                    nc.tensor.matmul(acc[ds(32 * q, 16), ts(7, 64)],
                                     te_ap(g0 + 8), w_ap(jj0 + 7, 2),
                                     start=False, stop=(q == ngroups - 1),
                                     tile_position=tp, skip_group_check=True)
        # One full-width drain: rows 32q..32q+16 hold strip q's outputs,
        # other rows are garbage (shipped; host ignores them).
        rows = 32 * ngroups
        ob = opool.tile([rows, 512], DT_OUT, name="ob", tag="ob",
                        padded_shape=[128, 512])
        nc.vector.tensor_copy(ob[:], acc[ds(0, rows), :])
        nc.gpsimd.dma_start(out_d[ds(0, rows), ds(h * 512, 512)], ob[:])
        s += n


def _build_nc(n_iters=None):
    """n_iters=None: straight-line kernel (graded path).
    n_iters=N: body wrapped in a HW For_i loop, for timing-slope runs."""
    nc = bacc.Bacc("TRN2", target_bir_lowering=False, debug=False)

    w_d = nc.declare_dram_parameter("wd", [128, W_TOT], DT_W, isOutput=False)
    x_d = nc.declare_dram_parameter("xd", [128, X_TOT], DT_X, isOutput=False)
    # out[p, h*512 + j*64 + f]: p = 32*q + phase*8 + b (rows 32q+16..32q+32
    # garbage), block h strip q covers pairs P0(h) + 8q .. +8q+7.
    out_d = nc.declare_dram_parameter("out", [128, NB * 512], DT_OUT,
                                      isOutput=True)

    with tile.TileContext(nc) as tc:
        with (
            tc.tile_pool(name="wpool", bufs=6) as wpool,
            tc.tile_pool(name="xpool", bufs=6) as xpool,
            tc.tile_pool(name="opool", bufs=4) as opool,
            tc.tile_pool(name="pspool", bufs=4, space=bass.MemorySpace.PSUM) as pspool,
        ):
            if n_iters is None:
                _build_body(nc, wpool, xpool, opool, pspool, w_d, x_d, out_d)
            else:
                with tc.For_i(0, n_iters, 1):
                    _build_body(nc, wpool, xpool, opool, pspool, w_d, x_d,
                                out_d)

    nc.compile()
    return nc


def _prep_inputs(x, kernel):
    """Host-side rearrangement into per-core per-block blobs."""
    xp = np.zeros((B, L + 4, C), np.float32)
    xp[:, :L] = x
    kp = np.zeros((N_CORES * P_CORE, K, C, F), np.float32)
    kp[:L_OUT] = kernel
    in_maps = []
    for m in range(N_CORES):
        l0 = P_CORE * m
        xs = xp[:, l0:l0 + 2 * PAIRS + 2, :]
        ev = xs[:, 0::2].transpose(2, 1, 0)  # (64, 257, 8)  position 2i
        od = xs[:, 1::2].transpose(2, 1, 0)  # (64, 257, 8)  position 2i+1
        # TE[i]: block-diag(x[2i], x[2i+1]) as (128, 16)
        TE = np.zeros((128, PAIRS + 1, 16), np.float32)
        TE[:64, :, 0:8] = ev
        TE[64:, :, 8:16] = od
        W = (kp[l0:l0 + P_CORE]
             .reshape(PAIRS, 2, K, C, F)
             .transpose(1, 3, 0, 2, 4)
             .reshape(128, PAIRS, K, F))  # [pc, pair, k, f]
        wb = np.empty((128, W_TOT), np.float32)
        xb = np.empty((128, X_TOT), np.float32)
        s = 0
        for h, n in enumerate(BLOCKS):
            wb[:, W_OFF[h]:W_OFF[h] + _w_cols(n)] = (
                W[:, s:s + n].reshape(128, _w_cols(n)))
            xb[:, X_OFF[h]:X_OFF[h] + _te_cols(n)] = (
                TE[:, s:s + n + 1].reshape(128, _te_cols(n)))
            s += n
        in_maps.append({
            "wd": (wb * WSCALE).astype(NP_W),
            "xd": xb.astype(NP_X),
        })
    return in_maps


def _unpack_out(res):
    """(128, NB*512) per core -> (B, P_CORE, F).

    res[32q + 16*ph8 ... p = 32q + phase*8 + b, h*512 + j*64 + f];
    l_local = 2*(P0(h) + 8q + j) + phase."""
    r = res.astype(np.float32).reshape(128, NB, 8, 64)  # [p, h, j, f]
    out = np.empty((B, P_CORE, F), np.float32)
    P0 = np.cumsum([0] + BLOCKS).tolist()
    for h, n in enumerate(BLOCKS):
        for q in range(n // 8):
            blk = r[32 * q:32 * q + 16, h].reshape(2, 8, 8, 64)  # ph, b, j, f
            for ph in range(2):
                ls = 2 * (P0[h] + 8 * q) + ph
                # positions ls, ls+2, ..., ls+14  (j = 0..7)
                out[:, ls:ls + 16:2, :] = blk[ph]
    return out


def kernel(x, kernel, bias):
    x = np.asarray(x, dtype=np.float32)
    kern = np.asarray(kernel, dtype=np.float32)
    bias = np.asarray(bias, dtype=np.float32)

    if "nc" not in _CACHE:
        _CACHE["nc"] = _build_nc()
    nc = _CACHE["nc"]

    in_maps = _prep_inputs(x, kern)
    results = run_bass_kernel_spmd(nc, in_maps, list(range(N_CORES))).results

    parts = [_unpack_out(results[m]["out"]) for m in range(N_CORES)]
    out = np.concatenate(parts, axis=1)[:, :L_OUT] * (1.0 / WSCALE)
    return (out + bias[None]).astype(np.float32)


# revision 15
# speedup vs baseline: 1.5229x; 1.0713x over previous
"""LocallyConnected1D (B=8, L=4096, C=64, K=3, F=64) on 8 TRN2 NeuronCores.

out[b, l, f] = sum_{k,c} x[b, l+k, c] * kernel[l, k, c, f] + bias[l, f]

Strategy (spatial sharding, 512 output positions per core):
  - Pairs of adjacent output positions (2i, 2i+1): stationary tile TE[i]
    (128 x 16) = block-diag(x[2i], x[2i+1]) over (2 phases x 64 channels)
    partitions; streaming operand = per-position fp8 weights.  TO[i] =
    block-diag(x[2i+1], x[2i+2]) serves tap 1.
  - Per pair: tap1 (TO, 64-col matmul) opens the pair, then TE matmuls
    accumulate.  TE[j] serves pair j-1 tap2 AND pair j tap0 whose weight and
    PSUM columns are adjacent -> fused 128-col matmuls (17 MM per group of 8
    pairs instead of 24).  PSUM has_written bits: only the block's first MM
    uses start=True (clears the whole bank); bit=0 -> overwrite handles each
    fresh region after that.
  - All 4 groups of a 32-pair block accumulate into ONE PSUM bank (strip q ->
    partitions 32q..32q+16, tile_position col 32q) so a single full-width
    [128, 512] DVE copy drains the block (vs 4 thin 16-row copies).
  - HBM traffic minimized: weights ship as fp8 e3m4 (x16 host prescale, ~1.4e-2
    max-rel error), x ships as bf16 TE tiles only; TO tiles are built on-chip
    by two partition-shifted DVE copies whose column shift (+8) picks up TE's
    own zero quadrants, so no memsets are needed.  Output returns as bf16.
  - Per block: 2 input DMAs (sync ring), 1 output DMA (scalar ring) -> the
    ~600ns-per-DMA sequencer issue cost stays off the critical path.
"""

import numpy as np
import ml_dtypes

import concourse.bass as bass
import concourse.mybir as mybir
import concourse.tile as tile
from concourse import bacc
from concourse.bass import ds, ts
from concourse.bass_utils import run_bass_kernel_spmd

B, L, C, K, F = 8, 4096, 64, 3, 64
L_OUT = (L - K) + 1  # 4094
N_CORES = 8
P_CORE = 512          # output positions per core (last core: 510 real + 2 pad)
PAIRS = P_CORE // 2   # 256

BLOCKS = [8, 8, 32, 32, 32, 32, 32, 32, 32, 16]   # pairs per block
assert sum(BLOCKS) == PAIRS and all(b % 8 == 0 for b in BLOCKS)
NB = len(BLOCKS)

WSCALE = 16.0  # host-side weight prescale before fp8 e3m4 cast
DT_W = mybir.dt.float8e3
NP_W = ml_dtypes.float8_e3m4
DT_X = mybir.dt.bfloat16
NP_X = ml_dtypes.bfloat16
DT_OUT = mybir.dt.bfloat16


def _w_cols(n):
    return n * K * F


def _te_cols(n):
    return (n + 1) * 16


W_OFF = np.cumsum([0] + [_w_cols(n) for n in BLOCKS]).tolist()
X_OFF = np.cumsum([0] + [_te_cols(n) for n in BLOCKS]).tolist()
W_TOT = W_OFF[-1]
X_TOT = X_OFF[-1]

_CACHE = {}


def _build_body(nc, wpool, xpool, opool, pspool, w_d, x_d, out_d):
    s = 0  # first pair of current block
    for h, n in enumerate(BLOCKS):
        te_c = _te_cols(n)
        to_c = n * 16
        wblk = wpool.tile([128, _w_cols(n)], DT_W, name="wblk", tag="wblk",
                          padded_shape=[128, _w_cols(max(BLOCKS))])
        xblk = xpool.tile([128, te_c + to_c], DT_X, name="xblk", tag="xblk",
                          padded_shape=[128, _te_cols(max(BLOCKS)) +
                                        max(BLOCKS) * 16])
        # Two HWDGE rings (SP=sync, ACT=scalar) each serialize their own
        # transfers; alternate w/x across them so each ring carries ~half the
        # input bytes.  Output DMAs ride the otherwise-idle SWDGE (gpsimd)
        # path so they never head-of-line-block an input ring.
        wq = nc.sync if h % 2 else nc.scalar
        xq = nc.scalar if h % 2 else nc.sync
        wq.dma_start(wblk[:], w_d[:, ds(W_OFF[h], _w_cols(n))])
        xq.dma_start(xblk[:, ds(0, te_c)], x_d[:, ds(X_OFF[h], te_c)])
        # Build TO tiles from TE tiles: TO[j] = blockdiag(od[j], ev[j+1]).
        # The +8 column shift reads TE's zero quadrants into TO's, so the
        # whole TO region (including zeros) is written.
        nc.vector.tensor_copy(xblk[ds(0, 64), ds(te_c, to_c)],
                              xblk[ds(64, 64), ds(8, to_c)])
        nc.vector.tensor_copy(xblk[ds(64, 64), ds(te_c, to_c)],
                              xblk[ds(0, 64), ds(8, to_c)])

        ngroups = n // 8
        acc = pspool.tile([128, 512], mybir.dt.float32, name="acc", tag="acc")

        def te_ap(i):   # block-diag tile for even-start pair i (global idx)
            return xblk[:, ds((i - s) * 16, 16)]

        def to_ap(i):   # odd-start pair i
            return xblk[:, ds(te_c + (i - s) * 16, 16)]

        def w_ap(jj, k, w=F):
            return wblk[:, ds((jj * K + k) * F, w)]

        # Per group (strip q): TO[0] TE[0] TO[1] TE[1] ... TO[7] TE[7] TE[8].
        # start=True only on the very first MM of the block (clears the whole
        # bank's has_written bits, stale from the pool's previous use).
        for step in range(17):
            for q in range(ngroups):
                g0 = s + q * 8      # first global pair of this group
                jj0 = q * 8         # first in-block pair of this group
                tp = (0, 32 * q)
                j, ph = divmod(step, 2)
                if ph == 0 and j < 8:     # TO[j]: pair j tap1, opens the pair
                    nc.tensor.matmul(acc[ds(32 * q, 16), ts(j, 64)],
                                     to_ap(g0 + j), w_ap(jj0 + j, 1),
                                     start=(step == 0), stop=False,
                                     tile_position=tp, skip_group_check=True)
                elif ph == 1 and j < 8:   # TE[j]
                    if j == 0:            # leading edge: pair 0 tap0 only
                        nc.tensor.matmul(acc[ds(32 * q, 16), ts(0, 64)],
                                         te_ap(g0), w_ap(jj0, 0),
                                         start=False, stop=False,
                                         tile_position=tp,
                                         skip_group_check=True)
                    else:                 # fused: pair j-1 tap2 | pair j tap0
                        nc.tensor.matmul(acc[ds(32 * q, 16),
                                             ds((j - 1) * 64, 128)],
                                         te_ap(g0 + j),
                                         w_ap(jj0 + j - 1, 2, 128),
                                         start=False, stop=False,
                                         tile_position=tp,
                                         skip_group_check=True)
                else:                     # step 16: trailing TE[8], pair7 tap2
                    nc.tensor.matmul(acc[ds(32 * q, 16), ts(7, 64)],
                                     te_ap(g0 + 8), w_ap(jj0 + 7, 2),
                                     start=False, stop=(q == ngroups - 1),
                                     tile_position=tp, skip_group_check=True)
        # One full-width drain: rows 32q..32q+16 hold strip q's outputs,
        # other rows are garbage (shipped; host ignores them).
        rows = 32 * ngroups
        ob = opool.tile([rows, 512], DT_OUT, name="ob", tag="ob",
                        padded_shape=[128, 512])
        nc.vector.tensor_copy(ob[:], acc[ds(0, rows), :])
        nc.scalar.dma_start(out_d[ds(0, rows), ds(h * 512, 512)], ob[:])
        s += n


def _build_nc(n_iters=None):
    """n_iters=None: straight-line kernel (graded path).
    n_iters=N: body wrapped in a HW For_i loop, for timing-slope runs."""
    nc = bacc.Bacc("TRN2", target_bir_lowering=False, debug=False)

    w_d = nc.declare_dram_parameter("wd", [128, W_TOT], DT_W, isOutput=False)
    x_d = nc.declare_dram_parameter("xd", [128, X_TOT], DT_X, isOutput=False)
    # out[p, h*512 + j*64 + f]: p = 32*q + phase*8 + b (rows 32q+16..32q+32
    # garbage), block h strip q covers pairs P0(h) + 8q .. +8q+7.
    out_d = nc.declare_dram_parameter("out", [128, NB * 512], DT_OUT,
                                      isOutput=True)

    with tile.TileContext(nc) as tc:
        with (
            tc.tile_pool(name="wpool", bufs=6) as wpool,
            tc.tile_pool(name="xpool", bufs=6) as xpool,
            tc.tile_pool(name="opool", bufs=4) as opool,
            tc.tile_pool(name="pspool", bufs=4, space=bass.MemorySpace.PSUM) as pspool,
        ):
            if n_iters is None:
                _build_body(nc, wpool, xpool, opool, pspool, w_d, x_d, out_d)
            else:
                with tc.For_i(0, n_iters, 1):
                    _build_body(nc, wpool, xpool, opool, pspool, w_d, x_d,
                                out_d)

    nc.compile()
    return nc


def _prep_inputs(x, kernel):
    """Host-side rearrangement into per-core per-block blobs."""
    xp = np.zeros((B, L + 4, C), np.float32)
    xp[:, :L] = x
    kp = np.zeros((N_CORES * P_CORE, K, C, F), np.float32)
    kp[:L_OUT] = kernel
    in_maps = []
    for m in range(N_CORES):
        l0 = P_CORE * m
        xs = xp[:, l0:l0 + 2 * PAIRS + 2, :]
        ev = xs[:, 0::2].transpose(2, 1, 0)  # (64, 257, 8)  position 2i
        od = xs[:, 1::2].transpose(2, 1, 0)  # (64, 257, 8)  position 2i+1
        # TE[i]: block-diag(x[2i], x[2i+1]) as (128, 16)
        TE = np.zeros((128, PAIRS + 1, 16), np.float32)
        TE[:64, :, 0:8] = ev
        TE[64:, :, 8:16] = od
        W = (kp[l0:l0 + P_CORE]
             .reshape(PAIRS, 2, K, C, F)
             .transpose(1, 3, 0, 2, 4)
             .reshape(128, PAIRS, K, F))  # [pc, pair, k, f]
        wb = np.empty((128, W_TOT), np.float32)
        xb = np.empty((128, X_TOT), np.float32)
        s = 0
        for h, n in enumerate(BLOCKS):
            wb[:, W_OFF[h]:W_OFF[h] + _w_cols(n)] = (
                W[:, s:s + n].reshape(128, _w_cols(n)))
            xb[:, X_OFF[h]:X_OFF[h] + _te_cols(n)] = (
                TE[:, s:s + n + 1].reshape(128, _te_cols(n)))
            s += n
        in_maps.append({
            "wd": (wb * WSCALE).astype(NP_W),
            "xd": xb.astype(NP_X),
        })
    return in_maps


def _unpack_out(res):
    """(128, NB*512) per core -> (B, P_CORE, F).

    res[32q + 16*ph8 ... p = 32q + phase*8 + b, h*512 + j*64 + f];
    l_local = 2*(P0(h) + 8q + j) + phase."""
    r = res.astype(np.float32).reshape(128, NB, 8, 64)  # [p, h, j, f]
    out = np.empty((B, P_CORE, F), np.float32)
    P0 = np.cumsum([0] + BLOCKS).tolist()
    for h, n in enumerate(BLOCKS):
        for q in range(n // 8):
            blk = r[32 * q:32 * q + 16, h].reshape(2, 8, 8, 64)  # ph, b, j, f
            for ph in range(2):
                ls = 2 * (P0[h] + 8 * q) + ph
                # positions ls, ls+2, ..., ls+14  (j = 0..7)
                out[:, ls:ls + 16:2, :] = blk[ph]
    return out


def kernel(x, kernel, bias):
    x = np.asarray(x, dtype=np.float32)
    kern = np.asarray(kernel, dtype=np.float32)
    bias = np.asarray(bias, dtype=np.float32)

    if "nc" not in _CACHE:
        _CACHE["nc"] = _build_nc()
    nc = _CACHE["nc"]

    in_maps = _prep_inputs(x, kern)
    results = run_bass_kernel_spmd(nc, in_maps, list(range(N_CORES))).results

    parts = [_unpack_out(results[m]["out"]) for m in range(N_CORES)]
    out = np.concatenate(parts, axis=1)[:, :L_OUT] * (1.0 / WSCALE)
    return (out + bias[None]).astype(np.float32)
